# revision 16
# baseline (speedup 1.0000x reference)
"""VQ codebook encoding (nn_Encoding) Trainium2 Bass kernel.

Math (per batch b):
    xf = x[b].reshape(C, N).T                      # (N, C)
    logits[n,k] = scale_k * (||xf_n||^2 - 2 xf_n.cw_k + ||cw_k||^2)
    w = softmax(logits, axis=k)
    enc[k,:]  = sum_n w[n,k] * (xf_n - cw_k)

Device decomposition (data-parallel over batch, 2 batches/core on 8 cores),
all matmuls in bf16 (1 cycle/col on PE + fast weight loads; fp32r pays
4 cycles/col below 256 moving cols on TRN2):

    Per 128-pixel chunk, each x c-chunk [128c, 128n] is loaded ONCE as the
    PE stationary operand and used twice:
      - transpose:  xT chunk [n, c]  = x_cc.T @ I          (128 cols, bf16 PSUM)
      - mm1:        logits [n, K]   += x_cc.T @ at_cc      (32 cols), with
                    at[c,k] = -2 scale_k cw[k,c]
    A tiny 2-row stationary [s[n]; 1] against rhs [scale_k; scale_k c2_k]
    adds the remaining scale_k*(||x_n||^2 + c2_k) rank-2 term (s = rowsum of
    x^2 is host-side input prep, uploaded as 16 KB).
    Softmax in [n, K] layout: one grouped ACT Exp per 4 chunks (PSUM source),
    denominators via DVE tensor_scalar accum_out (free-dim reduction),
    weights w = numer/denom in bf16.
    mm2: enc[K, C] += w.T @ xT per chunk (512 cols); w-sums via a -1s rhs;
    final enc += wsum*cw on DVE (scalar_tensor_tensor).

    xT PSUM->SBUF copies alternate DVE/ACT to balance engine load. mm2 for
    group g is emitted after transposes of group g+1 so the PE never stalls
    on the softmax cross-engine latency.

x is uploaded as bf16 (halves HBM traffic; error budget is generous: the
softmax is near-one-hot for this problem family and logits are <= 0, so
un-maxed Exp underflow is harmless). End-to-end rel err ~1e-3 vs fp32.
"""
import os
import numpy as np

B, C, N, K = 16, 512, 4096, 32
NCORES = 8
BPC = B // NCORES          # batches per core
CC = C // 128              # c chunks
NCH = N // 128             # n chunks per batch (32)
GRP = 4                    # n chunks per softmax/exp group
NGRP = NCH // GRP          # groups per batch (8)
SEG = 1024                 # pixels per x DMA segment
NSEG = N // SEG

_CACHE = {}


def _patch_tile_drain(tile, mybir, ScopedClock):
    """This walrus build rejects any instruction carrying >1 sync wait.
    Split extra waits onto single-wait NoOps on the same engine."""
    if getattr(tile.TileContext, "_multiwait_patched", False):
        return
    tile.TileContext._multiwait_patched = True

    _orig_add = tile.TileContext._add_instruction

    def _split_add(self, inst):
        si = inst.sync_info
        if si is not None and si.on_wait and len(si.on_wait) > 1:
            waits = list(si.on_wait)
            for w in waits[:-1]:
                nop = mybir.InstNoOp(name=f"waitnop-{self.nc.next_id()}", ins=[], outs=[])
                nop.engine = inst.engine
                nop.sync_info = mybir.SyncInfo(on_wait=[w], on_update=[])
                _orig_add(self, nop)
            si.on_wait = [waits[-1]]
            inst.sync_info = si
        _orig_add(self, inst)

    tile.TileContext._add_instruction = _split_add

    def _patched_drain(self, tick_clock, wait_clock):
        nc = self.nc
        probe = nc.sync.drain()
        wait_clock.add_sem_waits(probe.ins, ScopedClock({None: tick_clock.global_clock}))
        raw = probe.ins
        waits = list(raw.sync_info.on_wait) if raw.sync_info and raw.sync_info.on_wait else []
        if raw.sync_info is not None:
            raw.sync_info.on_wait = []
        for w in waits:
            wi = nc.sync.nop()
            wi.ins.sync_info = mybir.SyncInfo(on_wait=[w], on_update=[])
        nc.all_engine_barrier()
        assert self.sems is not None
        popped = nc._tile_sem_poison_stack.pop()
        assert popped is self._sem_poison
        nc.clear_and_free_semaphores(list(self.sems.allocated().values()))
        nc.all_engine_barrier()

    tile.TileContext._drain_and_barrier = _patched_drain


def _build():
    import concourse.bass as bass
    import concourse.tile as tile
    from concourse import mybir
    from concourse.vector_clock import ScopedClock

    _patch_tile_drain(tile, mybir, ScopedClock)

    F32 = mybir.dt.float32
    BF16 = mybir.dt.bfloat16
    Alu = mybir.AluOpType
    Act = mybir.ActivationFunctionType

    nc = bass.Bass("TRN2", target_bir_lowering=False, debug=False, num_devices=NCORES)
    x_ext = nc.dram_tensor("x", [BPC, CC, 128, N], BF16, kind="ExternalInput").ap()
    at_ext = nc.dram_tensor("at", [128, CC, K], BF16, kind="ExternalInput").ap()
    s2_ext = nc.dram_tensor("s2", [32, BPC, NCH, 128], BF16, kind="ExternalInput").ap()
    sc2_ext = nc.dram_tensor("sc2", [32, K], BF16, kind="ExternalInput").ap()
    cw_ext = nc.dram_tensor("cw", [K, C], F32, kind="ExternalInput").ap()
    id_ext = nc.dram_tensor("ident", [128, 128], BF16, kind="ExternalInput").ap()
    sel_ext = nc.dram_tensor("sel", [GRP * K, K], BF16, kind="ExternalInput").ap()
    enc_ext = nc.dram_tensor("enc", [BPC, K, C], F32, kind="ExternalOutput").ap()

    with tile.TileContext(nc) as tc:
        with (
            tc.tile_pool(name="singles", bufs=1) as singles,
            tc.tile_pool(name="xin", bufs=2) as xin,
            tc.tile_pool(name="xts", bufs=3 * GRP) as xts,
            tc.tile_pool(name="small", bufs=3) as small,
            tc.tile_pool(name="outp", bufs=2) as outp,
            tc.tile_pool(name="ps_xt", bufs=4, space="PSUM") as ps_xt,
            tc.tile_pool(name="ps_lg", bufs=2, space="PSUM") as ps_lg,
            tc.tile_pool(name="ps_enc", bufs=1, space="PSUM") as ps_enc,
            tc.tile_pool(name="ps_ws", bufs=1, space="PSUM") as ps_ws,
        ):
            id_sb = singles.tile([128, 128], BF16)
            nc.gpsimd.dma_start(out=id_sb, in_=id_ext)
            at_sb = singles.tile([128, CC, K], BF16)
            nc.gpsimd.dma_start(out=at_sb, in_=at_ext)
            s2_sb = singles.tile([32, BPC, NCH, 128], BF16)
            nc.gpsimd.dma_start(out=s2_sb, in_=s2_ext)
            sc2_sb = singles.tile([32, K], BF16)
            nc.gpsimd.dma_start(out=sc2_sb, in_=sc2_ext)
            cw_sb = singles.tile([K, C], F32)
            nc.gpsimd.dma_start(out=cw_sb, in_=cw_ext)
            sel_sb = singles.tile([GRP * K, K], BF16)
            nc.gpsimd.dma_start(out=sel_sb, in_=sel_ext)
            negones_f = singles.tile([128, 4], F32)
            nc.vector.memset(negones_f, -1.0)
            negones = singles.tile([128, 4], BF16)
            nc.vector.tensor_copy(out=negones, in_=negones_f)

            # ---- x segments, all on the sync HWDGE ring; a small first
            # segment lets the PE start ~2 us earlier.
            seg_bounds = [0, 256, 1280, 2560, N]   # px offsets per batch
            xsegs = {}

            def seg_of(n0):
                for si in range(len(seg_bounds) - 1):
                    if n0 < seg_bounds[si + 1]:
                        return si
                raise AssertionError

            for b in range(BPC):
                for si in range(4):
                    lo, hi = seg_bounds[si], seg_bounds[si + 1]
                    xs = xin.tile([128, CC, hi - lo], BF16, tag=f"x{si}")
                    nc.sync.dma_start(
                        out=xs,
                        in_=x_ext[b][:, :, lo:hi].rearrange("cc p n -> p cc n"))
                    xsegs[(b, si)] = (xs, lo, hi)

            copy_tick = 0
            for b in range(BPC):
                enc_ps = ps_enc.tile([K, C], F32, tag="enc")
                ws_ps = ps_ws.tile([GRP * K, 8], F32, tag="ws")
                mm2_queue = []      # deferred (wt, jj, xt_sb) awaiting mm2
                mm2_done = 0

                def flush_mm2(closing=False):
                    nonlocal mm2_done, mm2_queue
                    for qi, (qwt, qjj, qxt) in enumerate(mm2_queue):
                        last = closing and qi == len(mm2_queue) - 1
                        nc.tensor.matmul(enc_ps, qwt[:, qjj, :], qxt,
                                         start=(mm2_done == 0), stop=last,
                                         skip_group_check=True)
                        if qjj == 0:
                            # one ws matmul per group: wt viewed [128, GRP*K],
                            # out [GRP*K<=128, 8]; jj-blocks folded at the end
                            ws_last = closing and qi == len(mm2_queue) - GRP
                            nc.tensor.matmul(
                                ws_ps[:, 0:4],
                                qwt.rearrange("p g k -> p (g k)"), negones,
                                start=(mm2_done == 0), stop=ws_last,
                                skip_group_check=True)
                        mm2_done += 1
                    mm2_queue = []

                prev_soft = None
                def finish_softmax(pnumer, pxt):
                    dcols = small.tile([128, GRP], F32, tag="dc")
                    wt = small.tile([128, GRP, K], BF16, tag="wt")
                    for jj in range(GRP):
                        nc.vector.tensor_scalar(
                            out=wt[:, jj, :], in0=pnumer[:, jj, :],
                            scalar1=1.0, scalar2=0.0,
                            op0=Alu.mult, op1=Alu.add,
                            accum_out=dcols[:, jj:jj + 1])
                    rden = small.tile([128, GRP], F32, tag="rd")
                    nc.vector.reciprocal(rden, dcols)
                    for jj in range(GRP):
                        nc.vector.tensor_scalar_mul(
                            wt[:, jj, :], in0=wt[:, jj, :],
                            scalar1=rden[:, jj:jj + 1])
                    for jj in range(GRP):
                        mm2_queue.append((wt, jj, pxt[jj]))

                prev_soft = None
                for g in range(NGRP):
                    lg_ps = ps_lg.tile([128, GRP, K], F32, tag="lg")
                    numer = small.tile([128, GRP, K], BF16, tag="numer")
                    xt_list = []
                    for jj in range(GRP):
                        j = g * GRP + jj
                        nglob = j * 128
                        si = seg_of(nglob)
                        xs, lo, _ = xsegs[(b, si)]
                        n0 = nglob - lo
                        xt_sb = xts.tile([128, C], BF16, tag="xts")
                        xt_ps = ps_xt.tile([128, C], BF16, tag="xt")
                        # ---- transpose + mm1 on the shared stationary chunk ----
                        for cc in range(CC):
                            nc.tensor.transpose(
                                xt_ps[:, cc * 128:(cc + 1) * 128],
                                xs[:, cc, n0:n0 + 128],
                                id_sb,
                            )
                            nc.tensor.matmul(
                                lg_ps[:, jj, :], xs[:, cc, n0:n0 + 128],
                                at_sb[:, cc, :],
                                start=(cc == 0), stop=False,
                                skip_group_check=True,
                            )
                        # rank-2 rest: scale_k*s[n] + scale_k*c2_k
                        nc.tensor.matmul(
                            lg_ps[:, jj, :], s2_sb[:, b, j, :], sc2_sb,
                            start=False, stop=True, skip_group_check=True,
                        )
                        if copy_tick % 8 in (0, 3, 6):
                            nc.vector.tensor_copy(out=xt_sb, in_=xt_ps)
                        else:
                            nc.scalar.copy(out=xt_sb, in_=xt_ps)
                        copy_tick += 1
                        xt_list.append(xt_sb)
                    # ---- mm2 of group g-2 (w definitely ready) ----
                    flush_mm2()
                    # ---- numerators for this group (one grouped Exp) ----
                    nc.scalar.activation(
                        out=numer.rearrange("p g k -> p (g k)"),
                        in_=lg_ps.rearrange("p g k -> p (g k)"),
                        func=Act.Exp)
                    # ---- DVE softmax tail for the previous group (its Exp
                    # finished a whole group ago, so the DVE never blocks) ----
                    if prev_soft is not None:
                        finish_softmax(*prev_soft)
                    prev_soft = (numer, xt_list)
                # finish softmax of the final group, then close mm2
                finish_softmax(*prev_soft)
                flush_mm2(closing=True)
                # fold the 4 jj-blocks: copy ws col to SBUF, then
                # wsum[k] = sum_jj wscol[32*jj + k] via the 4-block identity
                wscol = small.tile([GRP * K, 1], BF16, tag="wscol")
                nc.vector.tensor_copy(out=wscol, in_=ws_ps[:, 0:1])
                wsum_ps = ws_ps[0:K, 4:5]
                nc.tensor.matmul(wsum_ps, sel_sb, wscol, start=True, stop=True,
                                 skip_group_check=True)
                # ---- final: enc += wsum * cw (wsum is negative) ----
                enc_sb = outp.tile([K, C], F32, tag="enc_out")
                nc.vector.scalar_tensor_tensor(
                    out=enc_sb, in0=cw_sb, scalar=wsum_ps[:, 0:1], in1=enc_ps,
                    op0=Alu.mult, op1=Alu.add)
                nc.sync.dma_start(out=enc_ext[b], in_=enc_sb)

    return nc


def kernel(x, codewords, scale):
    from concourse.bass_utils import run_bass_kernel_spmd
    import ml_dtypes

    x = np.ascontiguousarray(x, dtype=np.float32)
    codewords = np.ascontiguousarray(codewords, dtype=np.float32)
    scale = np.ascontiguousarray(scale, dtype=np.float32)

    if "nc" not in _CACHE:
        _CACHE["nc"] = _build()
    nc = _CACHE["nc"]

    # host-side prep: shard + layout + tiny derived tensors
    xr = x.reshape(B, C, N)
    at = (-2.0 * scale[:, None] * codewords).T.copy()          # [C, K]
    at = at.reshape(CC, 128, K).transpose(1, 0, 2)             # [128, cc, K]
    at = np.ascontiguousarray(at, dtype=ml_dtypes.bfloat16)
    c2 = (codewords.astype(np.float64) ** 2).sum(1).astype(np.float32)
    sc2 = np.zeros((32, K), dtype=np.float32)
    sc2[0], sc2[1] = scale, scale * c2
    sc2 = sc2.astype(ml_dtypes.bfloat16)                           # [32, K]
    ident = np.eye(128, dtype=ml_dtypes.bfloat16)
    sel = np.zeros((128, K), dtype=np.float32)
    for _jj in range(4):
        sel[_jj * K + np.arange(K), np.arange(K)] = 1.0
    sel = sel.astype(ml_dtypes.bfloat16)

    s_all = np.einsum("bcn,bcn->bn", xr, xr)                   # [B, N]

    in_maps = []
    for i in range(NCORES):
        xc = xr[i * BPC:(i + 1) * BPC]                         # [BPC, C, N]
        xb = np.ascontiguousarray(
            xc.reshape(BPC, CC, 128, N), dtype=ml_dtypes.bfloat16)
        sc = s_all[i * BPC:(i + 1) * BPC].reshape(BPC, NCH, 128)
        s2 = np.zeros((32, BPC, NCH, 128), dtype=np.float32)
        s2[0], s2[1] = sc, 1.0                                 # [32,BPC,NCH,128]
        s2 = np.ascontiguousarray(s2, dtype=ml_dtypes.bfloat16)
        in_maps.append({
            "x": xb, "at": at, "s2": s2, "sc2": sc2,
            "cw": codewords, "ident": ident, "sel": sel,
        })
    tmpdir = os.environ.get("BASS_PROF_DIR") or None
    res = run_bass_kernel_spmd(nc, in_maps, list(range(NCORES)), tmpdir=tmpdir)
    _CACHE["last_results"] = res
    out = np.concatenate([res.results[i]["enc"] for i in range(NCORES)], axis=0)
    return out.astype(np.float32)


# revision 17
# speedup vs baseline: 1.2881x; 1.2881x over previous
"""VQ codebook encoding (nn_Encoding) Trainium2 Bass kernel.

Math (per batch b):
    xf = x[b].reshape(C, N).T                      # (N, C)
    logits[n,k] = scale_k * (||xf_n||^2 - 2 xf_n.cw_k + ||cw_k||^2)
    w = softmax(logits, axis=k)
    enc[k,:]  = sum_n w[n,k] * (xf_n - cw_k)

Device decomposition (data-parallel over batch, 2 batches/core on 8 cores),
all matmuls in bf16 (1 cycle/col on PE + fast weight loads; fp32r pays
4 cycles/col below 256 moving cols on TRN2):

    Per 128-pixel chunk, each x c-chunk [128c, 128n] is loaded as the PE
    stationary operand twice:
      - transpose:  xT chunk [n, c]  = x_cc.T @ I          (128 cols, bf16 PSUM)
      - mm1:        logits [n, K]   += x_cc.T @ at_cc      (32 cols), with
                    at[c,k] = -2 scale_k cw[k,c]
    A tiny padded-32-row stationary [s[n]; 1; 0...] against rhs
    [scale_k; scale_k c2_k; 0...] adds the remaining scale_k*(||x_n||^2+c2_k)
    rank-2 term (s = rowsum of x^2 is host-side input prep, 16 KB upload).
    Softmax in [n, K] layout: one grouped ACT Exp per chunk-group straight
    from PSUM; denominators via DVE tensor_scalar accum_out (free-dim
    reduction); w = numer/denom in bf16. The DVE softmax tail runs one group
    deferred so it never blocks the copy stream; mm2 runs two groups
    deferred so the PE never waits on the cross-engine softmax chain.
    mm2: enc[K, C] += w.T @ xT per chunk (512 cols); w-sums via a -1s rhs
    column block; final enc += wsum*cw on DVE (scalar_tensor_tensor).
    xT PSUM->SBUF copies alternate DVE/ACT to balance engine load.
    The last groups of each batch shrink ([...,4,2,1,1]) so the end-of-batch
    softmax->mm2->store chain is short.

x is uploaded as bf16, c-major [BPC, CC, 128, N] (halves HBM traffic; the
error budget is generous: the softmax is near-one-hot for this problem
family and logits are <= 0, so un-maxed Exp underflow is harmless).
Constants are interleaved just-in-time with the first x segments on the
sync HWDGE ring. End-to-end rel err ~2e-3 vs the fp32 reference.
"""
import os
import numpy as np

B, C, N, K = 16, 512, 4096, 32
NCORES = 8
BPC = B // NCORES          # batches per core
CC = C // 128              # c chunks
NCH = N // 128             # n chunks per batch (32)
GROUP_PLAN = [4] * 7 + [2, 1, 1]       # chunk-groups per batch (sum 32)
SEG_BOUNDS = [0, 512, 1536, 2560, N]   # x DMA segment px offsets per batch

_CACHE = {}


def _patch_tile_drain(tile, mybir, ScopedClock):
    """This walrus build rejects any instruction carrying >1 sync wait.
    Split extra waits onto single-wait NoOps on the same engine."""
    if getattr(tile.TileContext, "_multiwait_patched", False):
        return
    tile.TileContext._multiwait_patched = True

    _orig_add = tile.TileContext._add_instruction

    def _split_add(self, inst):
        si = inst.sync_info
        if si is not None and si.on_wait and len(si.on_wait) > 1:
            waits = list(si.on_wait)
            for w in waits[:-1]:
                nop = mybir.InstNoOp(name=f"waitnop-{self.nc.next_id()}", ins=[], outs=[])
                nop.engine = inst.engine
                nop.sync_info = mybir.SyncInfo(on_wait=[w], on_update=[])
                _orig_add(self, nop)
            si.on_wait = [waits[-1]]
            inst.sync_info = si
        _orig_add(self, inst)

    tile.TileContext._add_instruction = _split_add

    def _patched_drain(self, tick_clock, wait_clock):
        nc = self.nc
        probe = nc.sync.drain()
        wait_clock.add_sem_waits(probe.ins, ScopedClock({None: tick_clock.global_clock}))
        raw = probe.ins
        waits = list(raw.sync_info.on_wait) if raw.sync_info and raw.sync_info.on_wait else []
        if raw.sync_info is not None:
            raw.sync_info.on_wait = []
        for w in waits:
            wi = nc.sync.nop()
            wi.ins.sync_info = mybir.SyncInfo(on_wait=[w], on_update=[])
        nc.all_engine_barrier()
        assert self.sems is not None
        popped = nc._tile_sem_poison_stack.pop()
        assert popped is self._sem_poison
        nc.clear_and_free_semaphores(list(self.sems.allocated().values()))
        nc.all_engine_barrier()

    tile.TileContext._drain_and_barrier = _patched_drain


def _build():
    import concourse.bass as bass
    import concourse.tile as tile
    from concourse import mybir
    from concourse.vector_clock import ScopedClock

    _patch_tile_drain(tile, mybir, ScopedClock)

    F32 = mybir.dt.float32
    BF16 = mybir.dt.bfloat16
    Alu = mybir.AluOpType
    Act = mybir.ActivationFunctionType

    nc = bass.Bass("TRN2", target_bir_lowering=False, debug=False, num_devices=NCORES)
    x_ext = nc.dram_tensor("x", [BPC, CC, 128, N], BF16, kind="ExternalInput").ap()
    at_ext = nc.dram_tensor("at", [128, CC, K], BF16, kind="ExternalInput").ap()
    s2_ext = nc.dram_tensor("s2", [32, BPC, NCH, 128], BF16, kind="ExternalInput").ap()
    sc2_ext = nc.dram_tensor("sc2", [32, K], BF16, kind="ExternalInput").ap()
    cw_ext = nc.dram_tensor("cw", [K, C], F32, kind="ExternalInput").ap()
    id_ext = nc.dram_tensor("ident", [128, 128], BF16, kind="ExternalInput").ap()
    enc_ext = nc.dram_tensor("enc", [BPC, K, C], F32, kind="ExternalOutput").ap()

    MAXG = max(GROUP_PLAN)

    with tile.TileContext(nc) as tc:
        with (
            tc.tile_pool(name="singles", bufs=1) as singles,
            tc.tile_pool(name="xin", bufs=2) as xin,
            tc.tile_pool(name="xts", bufs=3 * MAXG) as xts,
            tc.tile_pool(name="small", bufs=4) as small,
            tc.tile_pool(name="outp", bufs=2) as outp,
            tc.tile_pool(name="ps_xt", bufs=4, space="PSUM") as ps_xt,
            tc.tile_pool(name="ps_lg", bufs=2, space="PSUM") as ps_lg,
            tc.tile_pool(name="ps_enc", bufs=1, space="PSUM") as ps_enc,
            tc.tile_pool(name="ps_ws", bufs=1, space="PSUM") as ps_ws,
        ):
            # --- constants + x segments, just-in-time on the sync ring ---
            id_sb = singles.tile([128, 128], BF16)
            nc.sync.dma_start(out=id_sb, in_=id_ext)

            xsegs = {}

            def issue_seg(b, si):
                lo, hi = SEG_BOUNDS[si], SEG_BOUNDS[si + 1]
                xs = xin.tile([128, CC, hi - lo], BF16, tag=f"x{si}")
                nc.sync.dma_start(
                    out=xs,
                    in_=x_ext[b][:, :, lo:hi].rearrange("cc p n -> p cc n"))
                xsegs[(b, si)] = (xs, lo, hi)

            def seg_of(n0):
                for si in range(len(SEG_BOUNDS) - 1):
                    if n0 < SEG_BOUNDS[si + 1]:
                        return si
                raise AssertionError

            issue_seg(0, 0)
            at_sb = singles.tile([128, CC, K], BF16)
            nc.sync.dma_start(out=at_sb, in_=at_ext)
            sc2_sb = singles.tile([32, K], BF16)
            nc.sync.dma_start(out=sc2_sb, in_=sc2_ext)
            s2_sb = singles.tile([32, BPC, NCH, 128], BF16)
            nc.sync.dma_start(out=s2_sb, in_=s2_ext)
            for si in range(1, 4):
                issue_seg(0, si)
            for si in range(4):
                issue_seg(1, si)
            # cw is only needed at the end of batch 0; keep it off the ring
            cw_sb = singles.tile([K, C], F32)
            nc.gpsimd.dma_start(out=cw_sb, in_=cw_ext)
            negones_f = singles.tile([128, 8], F32)
            nc.vector.memset(negones_f, -1.0)
            negones = singles.tile([128, 8], BF16)
            nc.vector.tensor_copy(out=negones, in_=negones_f)

            copy_tick = 0
            for b in range(BPC):
                enc_ps = ps_enc.tile([K, C], F32, tag="enc")
                ws_ps = ps_ws.tile([K, 8], F32, tag="ws")
                mm2_queue = []      # deferred (wt, jj, xt_sb) awaiting mm2
                mm2_done = 0

                def flush_mm2(closing=False):
                    nonlocal mm2_done, mm2_queue
                    for qi, (qwt, qjj, qxt) in enumerate(mm2_queue):
                        last = closing and qi == len(mm2_queue) - 1
                        nc.tensor.matmul(enc_ps, qwt[:, qjj, :], qxt,
                                         start=(mm2_done == 0), stop=last,
                                         skip_group_check=True)
                        nc.tensor.matmul(ws_ps, qwt[:, qjj, :], negones,
                                         start=(mm2_done == 0), stop=last,
                                         skip_group_check=True)
                        mm2_done += 1
                    mm2_queue = []

                def finish_softmax(pnumer, pxt):
                    glen = len(pxt)
                    dcols = small.tile([128, MAXG], F32, tag="dc")
                    wt = small.tile([128, MAXG, K], BF16, tag="wt")
                    for jj in range(glen):
                        nc.vector.tensor_scalar(
                            out=wt[:, jj, :], in0=pnumer[:, jj, :],
                            scalar1=1.0, scalar2=0.0,
                            op0=Alu.mult, op1=Alu.add,
                            accum_out=dcols[:, jj:jj + 1])
                    rden = small.tile([128, MAXG], F32, tag="rd")
                    nc.vector.reciprocal(rden[:, 0:glen], dcols[:, 0:glen])
                    for jj in range(glen):
                        nc.vector.tensor_scalar_mul(
                            wt[:, jj, :], in0=wt[:, jj, :],
                            scalar1=rden[:, jj:jj + 1])
                    for jj in range(glen):
                        mm2_queue.append((wt, jj, pxt[jj]))

                prev_soft = None
                j = 0                      # chunk index within batch
                for glen in GROUP_PLAN:
                    lg_ps = ps_lg.tile([128, MAXG, K], F32, tag="lg")
                    numer = small.tile([128, MAXG, K], BF16, tag="numer")
                    xt_list = []
                    for jj in range(glen):
                        nglob = j * 128
                        si = seg_of(nglob)
                        xs, lo, _ = xsegs[(b, si)]
                        n0 = nglob - lo
                        xt_sb = xts.tile([128, C], BF16, tag="xts")
                        xt_ps = ps_xt.tile([128, C], BF16, tag="xt")
                        for cc in range(CC):
                            nc.tensor.transpose(
                                xt_ps[:, cc * 128:(cc + 1) * 128],
                                xs[:, cc, n0:n0 + 128],
                                id_sb,
                            )
                            nc.tensor.matmul(
                                lg_ps[:, jj, :], xs[:, cc, n0:n0 + 128],
                                at_sb[:, cc, :],
                                start=(cc == 0), stop=False,
                                skip_group_check=True,
                            )
                        # rank-2 rest: scale_k*s[n] + scale_k*c2_k
                        nc.tensor.matmul(
                            lg_ps[:, jj, :], s2_sb[:, b, j, :], sc2_sb,
                            start=False, stop=True, skip_group_check=True,
                        )
                        if copy_tick % 2 == 0:
                            nc.vector.tensor_copy(out=xt_sb, in_=xt_ps)
                        else:
                            nc.scalar.copy(out=xt_sb, in_=xt_ps)
                        copy_tick += 1
                        xt_list.append(xt_sb)
                        j += 1
                    # ---- mm2 of two groups back (w definitely ready) ----
                    flush_mm2()
                    # ---- numerators for this group (one grouped Exp) ----
                    nc.scalar.activation(
                        out=numer[:, 0:glen, :].rearrange("p g k -> p (g k)"),
                        in_=lg_ps[:, 0:glen, :].rearrange("p g k -> p (g k)"),
                        func=Act.Exp)
                    # ---- DVE softmax tail for the previous group ----
                    if prev_soft is not None:
                        finish_softmax(*prev_soft)
                    prev_soft = (numer, xt_list)
                # finish softmax of the final group, then close mm2
                finish_softmax(*prev_soft)
                flush_mm2(closing=True)
                # ---- final: enc += wsum * cw (wsum is negative) ----
                enc_sb = outp.tile([K, C], F32, tag="enc_out")
                nc.vector.scalar_tensor_tensor(
                    out=enc_sb, in0=cw_sb, scalar=ws_ps[:, 0:1], in1=enc_ps,
                    op0=Alu.mult, op1=Alu.add)
                nc.sync.dma_start(out=enc_ext[b], in_=enc_sb)

    return nc


def kernel(x, codewords, scale):
    from concourse.bass_utils import run_bass_kernel_spmd
    import ml_dtypes

    x = np.ascontiguousarray(x, dtype=np.float32)
    codewords = np.ascontiguousarray(codewords, dtype=np.float32)
    scale = np.ascontiguousarray(scale, dtype=np.float32)

    if "nc" not in _CACHE:
        _CACHE["nc"] = _build()
    nc = _CACHE["nc"]

    # host-side prep: shard + layout + tiny derived tensors
    xr = x.reshape(B, C, N)
    at = (-2.0 * scale[:, None] * codewords).T.copy()          # [C, K]
    at = at.reshape(CC, 128, K).transpose(1, 0, 2)             # [128, cc, K]
    at = np.ascontiguousarray(at, dtype=ml_dtypes.bfloat16)
    c2 = (codewords.astype(np.float64) ** 2).sum(1).astype(np.float32)
    sc2 = np.zeros((32, K), dtype=np.float32)
    sc2[0], sc2[1] = scale, scale * c2
    sc2 = sc2.astype(ml_dtypes.bfloat16)                       # [32, K]
    ident = np.eye(128, dtype=ml_dtypes.bfloat16)

    s_all = np.einsum("bcn,bcn->bn", xr, xr)                   # [B, N]

    in_maps = []
    for i in range(NCORES):
        xc = xr[i * BPC:(i + 1) * BPC]                         # [BPC, C, N]
        xb = np.ascontiguousarray(
            xc.reshape(BPC, CC, 128, N), dtype=ml_dtypes.bfloat16)
        sc = s_all[i * BPC:(i + 1) * BPC].reshape(BPC, NCH, 128)
        s2 = np.zeros((32, BPC, NCH, 128), dtype=np.float32)
        s2[0], s2[1] = sc, 1.0                                 # [32,BPC,NCH,128]
        s2 = np.ascontiguousarray(s2, dtype=ml_dtypes.bfloat16)
        in_maps.append({
            "x": xb, "at": at, "s2": s2, "sc2": sc2,
            "cw": codewords, "ident": ident,
        })
    tmpdir = os.environ.get("BASS_PROF_DIR") or None
    res = run_bass_kernel_spmd(nc, in_maps, list(range(NCORES)), tmpdir=tmpdir)
    _CACHE["last_results"] = res
    out = np.concatenate([res.results[i]["enc"] for i in range(NCORES)], axis=0)
    return out.astype(np.float32)
